# revision 36
# baseline (speedup 1.0000x reference)
"""Trainium2 Bass kernel for nn_AdvancedTransformer (axial + causal + local attention).

Strategy (8 NeuronCores, tensor-parallel over heads):
- Each core owns 2 of the 16 heads per attention layer.
- Data layout is "transposed": the carrier tensor xT is [D, S] so every
  projection matmul contracts D on the partition dim with no transposes.
- Flash-style softmax without max subtraction (scores are bounded ~|3.4|),
  row sums obtained for free via a ones-column appended to V in the P@V matmul.
- Causal/local masking via 0/1 mask multiplies after exp (host-precomputed),
  with band/triangle skipping of fully-masked work.
- Output projection produces per-core partial yT; an 8-core AllReduce (sum)
  combines them; residual + LayerNorm run replicated.
- LN gamma/beta are folded into the next layer's projection weights/biases on
  the host, so the on-device carrier is xhat (pre-affine LN output).
- Matmuls run in float32r (reduced-precision fp32, 4x faster; ~1e-4 rel err).
"""

import ml_dtypes
import numpy as np

import concourse.bass as bass
import concourse.mybir as mybir
import concourse.tile as tile
from concourse import bacc
from concourse.bass_utils import run_bass_kernel_spmd

F32 = mybir.dt.float32
F32R = mybir.dt.float32r
BF16 = mybir.dt.bfloat16
AF = mybir.ActivationFunctionType
ALU = mybir.AluOpType

B, S, D, H = 1, 2048, 1024, 16
HD = D // H          # 64
WIN = 256
N_CORES = 8
HPC = H // N_CORES   # heads per core = 2
P = 128
NKT = D // P         # 8 k-tiles over D
NST = S // P         # 16 s-tiles
NCH = S // 512       # 4 column chunks of 512
EPS = 1e-5
SCALE = float(1.0 / np.sqrt(HD))

LAYERS = [("ax", ["qr", "qc", "kr", "kc"]), ("ca", ["q", "k"]), ("lo", ["q", "k"])]


def _local_geom():
    """Per-stripe geometry for the local (banded) attention."""
    geo = []
    for i in range(NST):
        k0 = P * i
        qlo = min(max(k0 - 128, 0), S - 384)
        cmin, cmax = qlo // 512, (qlo + 383) // 512
        geo.append((qlo, cmin, cmax))
    contrib = {c: [i for i in range(NST) if geo[i][1] <= c <= geo[i][2]]
               for c in range(NCH)}
    return geo, contrib


def build_nc():
    nc = bacc.Bacc("TRN2", target_bir_lowering=False, debug=False,
                   num_devices=N_CORES)

    def din(name, shape, dt=F32R):
        return nc.dram_tensor(name, shape, dt, kind="ExternalInput").ap()

    # ---- inputs ----
    xT_d = din("xT", [D, S])
    ident_d = din("identity", [P, P], F32)
    tri_d = din("tri", [P, P], BF16)
    locmask_d = din("locmask", [NST, P, 384], BF16)
    ones_col_d = din("ones_col", [P, 1])
    ones128_d = din("ones128", [P, P])
    lay_in = {}
    for L, qks in LAYERS:
        for pn in qks + ["v"]:
            lay_in[f"{L}_W{pn}"] = din(f"{L}_W{pn}", [P, D])
        lay_in[f"{L}_pb"] = din(f"{L}_pb", [P, len(qks) + 1], F32)
        lay_in[f"{L}_Wo"] = din(f"{L}_Wo", [P, D])
        lay_in[f"{L}_bo"] = din(f"{L}_bo", [P, NKT], F32)
        lay_in[f"{L}_G"] = din(f"{L}_G", [P, NKT], F32)
        if L != "ax":
            lay_in[f"{L}_wsum"] = din(f"{L}_wsum", [1, (len(qks) + 1) * P])
            lay_in[f"{L}_negG"] = din(f"{L}_negG", [1, D])
    fin_g_d = din("fin_g", [P, NKT], F32)
    fin_b_d = din("fin_b", [P, NKT], F32)
    fin_grow_d = din("fin_grow", [1, D])

    out_d = nc.dram_tensor("out_xT", [D, S], F32, kind="ExternalOutput").ap()

    locgeo, loccontrib = _local_geom()

    with nc.allow_low_precision(reason="float32r matmul pipeline"), \
         tile.TileContext(nc) as tc:
        pX = tc.alloc_tile_pool(name="pX", bufs=NKT)
        pYF = tc.alloc_tile_pool(name="pYF", bufs=2)
        pQK = tc.alloc_tile_pool(name="pQK", bufs=4)
        pVT = tc.alloc_tile_pool(name="pVT", bufs=1)
        pPT = tc.alloc_tile_pool(name="pPT", bufs=2)
        pV = tc.alloc_tile_pool(name="pV", bufs=NST)
        pATT = tc.alloc_tile_pool(name="pATT", bufs=1)
        pW = tc.alloc_tile_pool(name="pW", bufs=3)
        pWO = tc.alloc_tile_pool(name="pWO", bufs=2)
        pMASK = tc.alloc_tile_pool(name="pMASK", bufs=3)
        pSQ = tc.alloc_tile_pool(name="pSQ", bufs=3)
        pYSB = tc.alloc_tile_pool(name="pYSB", bufs=3)
        pROW = tc.alloc_tile_pool(name="pROW", bufs=6)
        pZC = tc.alloc_tile_pool(name="pZC", bufs=10)
        pCONST = tc.alloc_tile_pool(name="pCONST", bufs=24)
        pPP = tc.alloc_tile_pool(name="pPP", bufs=4, space="PSUM")
        pOUT = tc.alloc_tile_pool(name="pOUT", bufs=1, space="PSUM")
        pDRAM = tc.alloc_tile_pool(name="pDRAM", bufs=2, space="DRAM")

        # ---- persistent loads ----
        X = []
        for i in range(NKT):
            xt = pX.tile([P, S], F32R, name=f"X{i}", tag="X")
            nc.sync.dma_start(out=xt[:, :], in_=xT_d[P * i:P * (i + 1), :])
            X.append(xt)
        ident = pCONST.tile([P, P], F32, name="ident")
        nc.sync.dma_start(out=ident[:, :], in_=ident_d[:, :])
        tri = pCONST.tile([P, P], BF16, name="tri")
        nc.sync.dma_start(out=tri[:, :], in_=tri_d[:, :])
        ones_col = pCONST.tile([P, 1], F32R, name="ones_col")
        nc.sync.dma_start(out=ones_col[:, :], in_=ones_col_d[:, :])
        ones128 = pCONST.tile([P, P], F32R, name="ones128")
        nc.sync.dma_start(out=ones128[:, :], in_=ones128_d[:, :])
        fin_g = pCONST.tile([P, NKT], F32, name="fin_g")
        nc.sync.dma_start(out=fin_g[:, :], in_=fin_g_d[:, :])
        fin_b = pCONST.tile([P, NKT], F32, name="fin_b")
        nc.sync.dma_start(out=fin_b[:, :], in_=fin_b_d[:, :])
        eps_t = pCONST.tile([1, 1], F32, name="eps_t")
        nc.vector.memset(eps_t[:, :], EPS)
        def proj_T(w_sb, bias_ap, out_sb, fold=None):
            """out_sb [128, S] = W^T @ carrier + b (transposed layout).
            With fold=(wsum_ap, mean_row, rstd_reps): carrier X holds z and
            the LN (z-mean)*rstd is folded in: psum = W^T z - wsum x mean,
            out = psum*rstd_rep + b."""
            for c in range(NCH):
                cs = slice(512 * c, 512 * (c + 1))
                ps = pPP.tile([P, 512], F32, tag="PP")
                last_kt = NKT - 1
                for kt in range(NKT):
                    nc.tensor.matmul(
                        ps[:, :], lhsT=w_sb[:, P * kt:P * (kt + 1)],
                        rhs=X[kt][:, cs],
                        start=(kt == 0),
                        stop=(fold is None and kt == last_kt))
                if fold is None:
                    nc.scalar.activation(out_sb[:, cs], ps[:, :],
                                         AF.Identity, bias=bias_ap, scale=1.0)
                else:
                    wsum_ap, mean_row, rstd_reps = fold
                    nc.tensor.matmul(ps[:, :], lhsT=wsum_ap,
                                     rhs=mean_row[0:1, cs],
                                     start=False, stop=True)
                    nc.vector.tensor_mul(out_sb[:, cs], ps[:, :],
                                         rstd_reps[c][:, :])
                    nc.vector.tensor_scalar_add(out_sb[:, cs], out_sb[:, cs],
                                                bias_ap)

        def row_attention(mode, qT, kT, v_tiles, h, attnT, layer_tag):
            """Standard attention for head h; writes normalized out into
            attnT[64h:64h+64, :]. mode in {"full", "causal", "local"}."""
            hp = slice(64 * h, 64 * h + 64)
            vcols = slice(65 * h, 65 * h + 65)  # [v_h | ones]
            outp = pOUT.tile([65, S], F32, tag="OUT",
                             name=f"{layer_tag}row{h}")
            for i in range(NST):
                k0 = P * i
                PT = pPT.tile([P, S], BF16, tag="PT")
                if mode == "full":
                    chunks = [(512 * c, 512 * (c + 1)) for c in range(NCH)]
                    pv_cs, first, last = list(range(NCH)), 0, NST - 1
                elif mode == "causal":
                    chunks = []
                    qs = k0
                    while qs < S:
                        qe = min((qs // 512 + 1) * 512, S)
                        chunks.append((qs, qe))
                        qs = qe
                    pv_cs = list(range(k0 // 512, NCH))
                else:  # local
                    qlo, cmin, cmax = locgeo[i]
                    chunks = [(qlo, qlo + 384)]
                    pv_cs = list(range(cmin, cmax + 1))
                for (qs, qe) in chunks:
                    sp = pPP.tile([P, qe - qs], F32, tag="PP")
                    nc.tensor.matmul(sp[:, :], lhsT=kT[hp, k0:k0 + P],
                                     rhs=qT[hp, qs:qe], start=True, stop=True)
                    nc.scalar.activation(PT[:, qs:qe], sp[:, :], AF.Exp,
                                         scale=SCALE)
                if mode == "causal":
                    nc.vector.tensor_mul(PT[:, k0:k0 + P], PT[:, k0:k0 + P],
                                         tri[:, :])
                    cb = 512 * (i // 4)
                    if k0 > cb:
                        nc.vector.memset(PT[:, cb:k0], 0.0)
                elif mode == "local":
                    qlo, cmin, cmax = locgeo[i]
                    nc.vector.tensor_mul(PT[:, qlo:qlo + 384],
                                         PT[:, qlo:qlo + 384],
                                         locmask_sb[:, 384 * i:384 * (i + 1)])
                    if qlo > 512 * cmin:
                        nc.vector.memset(PT[:, 512 * cmin:qlo], 0.0)
                    if qlo + 384 < 512 * (cmax + 1):
                        nc.vector.memset(PT[:, qlo + 384:512 * (cmax + 1)], 0.0)
                for c in pv_cs:
                    if mode == "full":
                        fi, la = 0, NST - 1
                    elif mode == "causal":
                        fi, la = 0, min(NST - 1, 4 * c + 3)
                    else:
                        fi, la = loccontrib[c][0], loccontrib[c][-1]
                    nc.tensor.matmul(outp[:, 512 * c:512 * (c + 1)],
                                     lhsT=v_tiles[i][:, vcols],
                                     rhs=PT[:, 512 * c:512 * (c + 1)],
                                     start=(i == fi), stop=(i == la))
            # normalize: out[hd, q] / Z[q]
            rz = pROW.tile([1, S], F32R, tag="ROW")
            nc.scalar.copy(rz[0:1, :], outp[64:65, :])
            nc.vector.reciprocal(rz[0:1, :], rz[0:1, :])
            for c in range(NCH):
                zrep = pPP.tile([64, 512], F32, tag="PP")
                nc.tensor.matmul(zrep[:, :], lhsT=ones128[0:1, 0:64],
                                 rhs=rz[0:1, 512 * c:512 * (c + 1)],
                                 start=True, stop=True)
                zrep_sb = pSQ.tile([64, 512], F32, tag="SQ")
                nc.scalar.copy(zrep_sb[:, :], zrep[:, :])
                nc.vector.tensor_mul(attnT[hp, 512 * c:512 * (c + 1)],
                                     outp[0:64, 512 * c:512 * (c + 1)],
                                     zrep_sb[:, :])

        def col_attention(qcT, kcT, v_tiles, h, attnT, layer_tag):
            """Axial column attention for head h; adds into attnT[64h:...]."""
            hp = slice(64 * h, 64 * h + 64)
            vdata = slice(65 * h, 65 * h + 64)  # v only (no ones col)
            colp = pOUT.tile([64, S], F32, tag="OUT", name=f"{layer_tag}col{h}")
            for r in range(NST):
                r0 = P * r
                PT = pPT.tile([P, S], BF16, tag="PT")
                zparts = []
                for c in range(NCH):
                    sp = pPP.tile([P, 512], F32, tag="PP")
                    nc.tensor.matmul(sp[:, :], lhsT=qcT[hp, r0:r0 + P],
                                     rhs=kcT[hp, 512 * c:512 * (c + 1)],
                                     start=True, stop=True)
                    zp = pZC.tile([P, 1], F32, tag="ZC")
                    nc.scalar.activation(PT[:, 512 * c:512 * (c + 1)], sp[:, :],
                                         AF.Exp, scale=SCALE, accum_out=zp[:, :])
                    zparts.append(zp)
                z01 = pZC.tile([P, 1], F32, tag="ZC")
                nc.vector.tensor_add(z01[:, :], zparts[0][:, :], zparts[1][:, :])
                z23 = pZC.tile([P, 1], F32, tag="ZC")
                nc.vector.tensor_add(z23[:, :], zparts[2][:, :], zparts[3][:, :])
                zs = pZC.tile([P, 1], F32, tag="ZC")
                nc.vector.tensor_add(zs[:, :], z01[:, :], z23[:, :])
                rzc = pZC.tile([P, 1], F32, tag="ZC")
                nc.vector.reciprocal(rzc[:, :], zs[:, :])
                vs = pZC.tile([P, 64], BF16, tag="VS", bufs=8)
                nc.vector.tensor_scalar_mul(vs[:, :], v_tiles[r][:, vdata],
                                            rzc[:, :])
                for c in range(NCH):
                    nc.tensor.matmul(colp[:, 512 * c:512 * (c + 1)],
                                     lhsT=vs[:, :],
                                     rhs=PT[:, 512 * c:512 * (c + 1)],
                                     start=(r == 0), stop=(r == NST - 1))
            for c in range(NCH):
                nc.vector.tensor_add(attnT[hp, 512 * c:512 * (c + 1)],
                                     attnT[hp, 512 * c:512 * (c + 1)],
                                     colp[:, 512 * c:512 * (c + 1)])

        # ================= layers =================
        # Carrier in X: raw x before layer 1, then z_n (pre-LN residual sum).
        # LN of z_{n-1} is folded into layer n's projections (mean via K=1
        # rank-1 matmul, rstd via epilogue scale); the residual x_{n-1} is
        # rebuilt during the X update: X = (G*X)*rstd_rep + yfull, where
        # yfull carries y + bo + B - G*row2 (constants injected on core 0).
        def emit_stats(L):
            mean_sb = pROW.tile([1, S], F32R, tag="ROW", name=f"{L}mean")
            msq_sb = pROW.tile([1, S], F32R, tag="ROW", name=f"{L}msq")
            wrow = pROW.tile([1, S], F32R, tag="ROW", name=f"{L}wrow")
            for c in range(NCH):
                cs = slice(512 * c, 512 * (c + 1))
                sps = pPP.tile([1, 512], F32, tag="PP", name=f"{L}sum{c}")
                for i in range(NKT):
                    nc.tensor.matmul(sps[:, :], lhsT=ones_col[:, :],
                                     rhs=X[i][:, cs], start=(i == 0),
                                     stop=(i == NKT - 1))
                nc.scalar.mul(mean_sb[0:1, cs], sps[:, :], 1.0 / D)
                sqs = pPP.tile([1, 512], F32, tag="PP", name=f"{L}sq{c}")
                for i in range(NKT):
                    sq = pSQ.tile([P, 512], F32R, tag="SQ")
                    eng = nc.gpsimd if i % 2 == 0 else nc.vector
                    eng.tensor_mul(sq[:, :], X[i][:, cs], X[i][:, cs])
                    nc.tensor.matmul(sqs[:, :], lhsT=ones_col[:, :],
                                     rhs=sq[:, :], start=(i == 0),
                                     stop=(i == NKT - 1))
                nc.scalar.mul(msq_sb[0:1, cs], sqs[:, :], 1.0 / D)
            nc.vector.tensor_mul(wrow[0:1, :], mean_sb[0:1, :], mean_sb[0:1, :])
            nc.vector.tensor_sub(msq_sb[0:1, :], msq_sb[0:1, :], wrow[0:1, :])
            nc.scalar.activation(wrow[0:1, :], msq_sb[0:1, :], AF.Sqrt,
                                 bias=eps_t[0:1, 0:1], scale=1.0)
            nc.vector.reciprocal(msq_sb[0:1, :], wrow[0:1, :])
            nc.vector.tensor_mul(wrow[0:1, :], mean_sb[0:1, :], msq_sb[0:1, :])
            rstd, row2 = msq_sb, wrow
            reps = []
            for c in range(NCH):
                cs = slice(512 * c, 512 * (c + 1))
                rp = pPP.tile([P, 512], F32, tag="PP", name=f"{L}rr{c}")
                nc.tensor.matmul(rp[:, :], lhsT=ones128[0:1, :],
                                 rhs=rstd[0:1, cs], start=True, stop=True)
                rs = pRREP.tile([P, 512], F32, tag="RREP", name=f"{L}rrs{c}")
                nc.scalar.copy(rs[:, :], rp[:, :])
                reps.append(rs)
            return mean_sb, reps, row2

        prev_stats = None
        for li, (L, qks) in enumerate(LAYERS):
            # -- load weights --
            wsb = {}
            for pn in qks + ["v"]:
                w = pW.tile([P, D], F32R, tag="W", name=f"{L}W{pn}")
                nc.sync.dma_start(out=w[:, :], in_=lay_in[f"{L}_W{pn}"][:, :])
                wsb[pn] = w
            wo = pWO.tile([P, D], F32R, tag="WO", name=f"{L}Wo")
            nc.sync.dma_start(out=wo[:, :], in_=lay_in[f"{L}_Wo"][:, :])
            pb = pCONST.tile([P, len(qks) + 1], F32, name=f"{L}pb")
            nc.sync.dma_start(out=pb[:, :], in_=lay_in[f"{L}_pb"][:, :])
            bo = pCONST.tile([P, NKT], F32, name=f"{L}bo")
            nc.sync.dma_start(out=bo[:, :], in_=lay_in[f"{L}_bo"][:, :])
            gg = pCONST.tile([P, NKT], F32, name=f"{L}G")
            nc.sync.dma_start(out=gg[:, :], in_=lay_in[f"{L}_G"][:, :])
            if prev_stats is not None:
                NP1 = len(qks) + 1
                wsum_sb = pCONST.tile([1, NP1 * P], F32R, name=f"{L}wsum")
                nc.sync.dma_start(out=wsum_sb[:, :],
                                  in_=lay_in[f"{L}_wsum"][:, :])
                negg_sb = pCONST.tile([1, D], F32R, name=f"{L}negG")
                nc.sync.dma_start(out=negg_sb[:, :],
                                  in_=lay_in[f"{L}_negG"][:, :])
                mean_sb, rstd_reps, row2 = prev_stats

            # -- projections (transposed, LN folded for layers 2+) --
            qkT = {}
            for j, pn in enumerate(qks):
                t = pQK.tile([P, S], BF16, tag="QK", name=f"{L}{pn}T")
                f = None if prev_stats is None else (
                    wsum_sb[0:1, P * j:P * (j + 1)], mean_sb, rstd_reps)
                proj_T(wsb[pn], pb[:, j:j + 1], t, fold=f)
                qkT[pn] = t
            # v: transposed projection then PE-transpose to natural
            jv = len(qks)
            vT = pVT.tile([P, S], F32, tag="VT", name=f"{L}vT")
            f = None if prev_stats is None else (
                wsum_sb[0:1, P * jv:P * (jv + 1)], mean_sb, rstd_reps)
            proj_T(wsb["v"], pb[:, jv:jv + 1], vT, fold=f)
            v_tiles = []
            for t in range(NST):
                pt = pPP.tile([P, P], F32, tag="PP", name=f"{L}vtr{t}")
                nc.tensor.transpose(pt[:, :], vT[:, P * t:P * (t + 1)],
                                    ident[:, :])
                vt = pV.tile([P, 65 * HPC], BF16, tag="V", name=f"{L}v{t}")
                for h in range(HPC):
                    nc.vector.tensor_copy(vt[:, 65 * h:65 * h + 64],
                                          pt[:, 64 * h:64 * h + 64])
                    nc.vector.memset(vt[:, 65 * h + 64:65 * h + 65], 1.0)
                v_tiles.append(vt)

            # -- attention --
            if L == "lo":
                locmask_sb = pMASK.tile([P, NST * 384], BF16, tag="MASK",
                                        name="locmask_sb")
                nc.sync.dma_start(
                    out=locmask_sb[:, :].rearrange("p (n q) -> p n q", n=NST),
                    in_=locmask_d[:, :, :].transpose([1, 0, 2]))
            attnT = pATT.tile([P, S], F32R, tag="ATT", name=f"{L}attnT")
            for h in range(HPC):
                if L == "ax":
                    row_attention("full", qkT["qr"], qkT["kr"], v_tiles, h,
                                  attnT, L)
                    col_attention(qkT["qc"], qkT["kc"], v_tiles, h, attnT, L)
                elif L == "ca":
                    row_attention("causal", qkT["q"], qkT["k"], v_tiles, h,
                                  attnT, L)
                else:
                    row_attention("local", qkT["q"], qkT["k"], v_tiles, h,
                                  attnT, L)

            # -- carrier transform (u-pass): X <- (G*X)*rstd_rep --
            # (turns z_{n-1} into x_{n-1} minus the constant terms, which
            # ride the collective below). Runs on DVE during attention/Wo.
            if prev_stats is not None:
                for i in range(NKT):
                    for c in range(NCH):
                        cs = slice(512 * c, 512 * (c + 1))
                        nc.vector.scalar_tensor_tensor(
                            out=X[i][:, cs], in0=X[i][:, cs],
                            scalar=gg[:, i:i + 1], in1=rstd_reps[c][:, :],
                            op0=ALU.mult, op1=ALU.mult)

            # -- output projection -> bf16 partial -> AllReduce --
            ybounce = pDRAM.tile([D, S], BF16, tag="YB", name=f"{L}yb")
            yfull = pDRAM.tile([D, S], BF16, tag="YFULL", name=f"{L}yf",
                               addr_space="Shared")
            for m in range(NKT):
                for c in range(NCH):
                    cs = slice(512 * c, 512 * (c + 1))
                    yp = pPP.tile([P, 512], F32, tag="PP")
                    nc.tensor.matmul(yp[:, :],
                                     lhsT=wo[:, P * m:P * (m + 1)],
                                     rhs=attnT[:, cs],
                                     start=True,
                                     stop=(prev_stats is None))
                    if prev_stats is not None:
                        # core-0 constant: -G (x) row2 (rank-1)
                        nc.tensor.matmul(yp[:, :],
                                         lhsT=negg_sb[0:1, P * m:P * (m + 1)],
                                         rhs=row2[0:1, cs],
                                         start=False, stop=True)
                    # PSUM->SBUF (bf16) + bias bo + B_prev (core 0)
                    y_sb = pYSB.tile([P, 512], BF16, tag="YSB")
                    if (m + c) % 2 == 0:
                        nc.vector.tensor_scalar_add(y_sb[:, :], yp[:, :],
                                                    bo[:, m:m + 1])
                    else:
                        nc.scalar.activation(y_sb[:, :], yp[:, :], AF.Identity,
                                             bias=bo[:, m:m + 1], scale=1.0)
                    nc.sync.dma_start(out=ybounce[P * m:P * (m + 1), cs],
                                      in_=y_sb[:, :])
            nc.gpsimd.collective_compute(
                "AllReduce", ALU.add,
                replica_groups=[list(range(N_CORES))],
                ins=[ybounce[:, :].opt()],
                outs=[yfull[:, :].opt()],
            )
            # -- X update: X <- X + yfull  (completes z_n) --
            for i in range(NKT):
                yf = pYF.tile([P, S], BF16, tag="YF")
                nc.sync.dma_start(out=yf[:, :],
                                  in_=yfull[P * i:P * (i + 1), :])
                nc.vector.tensor_add(X[i][:, :], X[i][:, :], yf[:, :])

            # -- stats of the new carrier z_n --
            prev_stats = emit_stats(L)

        # ================= final LN + affine =================
        # out = (z*rstd_rep)*g - g (x) row2  (B added on host), 2 DVE passes
        mean_sb, rstd_reps, row2 = prev_stats
        fin_grow = pW.tile([1, D], F32R, tag="W", name="fin_grow")
        nc.sync.dma_start(out=fin_grow[:, :], in_=fin_grow_d[:, :])
        for c in range(NCH):
            cs = slice(512 * c, 512 * (c + 1))
            for i in range(NKT):
                gr2 = pPP.tile([P, 512], F32, tag="PP", name=f"fg{i}_{c}")
                nc.tensor.matmul(gr2[:, :],
                                 lhsT=fin_grow[0:1, P * i:P * (i + 1)],
                                 rhs=row2[0:1, cs], start=True, stop=True)
                t1 = pSQ.tile([P, 512], F32, tag="SQ")
                nc.vector.tensor_mul(t1[:, :], X[i][:, cs],
                                     rstd_reps[c][:, :])
                nc.vector.scalar_tensor_tensor(
                    out=X[i][:, cs], in0=t1[:, :],
                    scalar=fin_g[:, i:i + 1], in1=gr2[:, :],
                    op0=ALU.mult, op1=ALU.subtract)
                nc.sync.dma_start(out=out_d[P * i:P * (i + 1), cs],
                                  in_=X[i][:, cs].bitcast(F32))

    nc.compile()
    return nc


def _pack(vec):
    """[1024] -> [128, 8] per-partition pack (col i = d-tile i)."""
    return np.ascontiguousarray(vec.reshape(NKT, P).T.astype(np.float32))


def _ktmajor(w):
    """[1024, 128] -> [128, 1024] with k-tile kt at cols [128kt:128kt+128)."""
    return np.ascontiguousarray(
        w.reshape(NKT, P, P).transpose(1, 0, 2).reshape(P, D).astype(np.float32))


def prep_in_maps(x, params):
    x = np.asarray(x, np.float32)
    pjax = {"ax": params["axial"], "ca": params["causal"], "lo": params["local"]}
    pp = {L: {k: np.asarray(v, np.float32) for k, v in pjax[L].items()}
          for L in pjax}

    idx = np.arange(S)
    common = {
        "xT": np.ascontiguousarray(x[0].T),
        "identity": np.eye(P, dtype=np.float32),
        "tri": (idx[:P, None] <= idx[None, :P]).astype(ml_dtypes.bfloat16),
        "ones_col": np.ones((P, 1), np.float32),
        "ones128": np.ones((P, P), np.float32),
    }
    locmask = np.zeros((NST, P, 384), np.float32)
    locgeo, _ = _local_geom()
    for i in range(NST):
        qlo, _, _ = locgeo[i]
        kk = P * i + idx[:P, None]
        qq = qlo + idx[None, :384]
        locmask[i] = (np.abs(kk - qq) <= WIN // 2).astype(np.float32)
    common["locmask"] = locmask.astype(ml_dtypes.bfloat16)

    in_maps = [dict(common) for _ in range(N_CORES)]
    G_prev = np.ones(D, np.float32)
    B_prev = np.zeros(D, np.float32)
    for L, qks in LAYERS:
        p = pp[L]
        names = {"ax": {"qr": "qr", "qc": "qc", "kr": "kr", "kc": "kc", "v": "v"},
                 "ca": {"q": "q", "k": "k", "v": "v"},
                 "lo": {"q": "q", "k": "k", "v": "v"}}[L]
        for r in range(N_CORES):
            hs = slice(r * P, (r + 1) * P)
            pbcols = []
            wsumcols = []
            for pn in qks + ["v"]:
                W = p["W" + names[pn]]
                bb = p["b" + names[pn]]
                Wf = W * G_prev[:, None]
                bf = bb + W.T @ B_prev
                in_maps[r][f"{L}_W{pn}"] = _ktmajor(Wf[:, hs])
                pbcols.append(bf[hs])
                wsumcols.append(-Wf[:, hs].sum(axis=0))
            in_maps[r][f"{L}_pb"] = np.ascontiguousarray(
                np.stack(pbcols, axis=1).astype(np.float32))
            in_maps[r][f"{L}_Wo"] = np.ascontiguousarray(p["Wo"][hs, :])
            bo_full = p["bo"] + B_prev
            in_maps[r][f"{L}_bo"] = (_pack(bo_full) if r == 0
                                     else np.zeros((P, NKT), np.float32))
            in_maps[r][f"{L}_G"] = _pack(G_prev)
            if L != "ax":
                in_maps[r][f"{L}_wsum"] = np.concatenate(
                    wsumcols).astype(np.float32)[None, :]
                in_maps[r][f"{L}_negG"] = ((-G_prev).astype(np.float32)[None, :]
                                           if r == 0
                                           else np.zeros((1, D), np.float32))
        G_prev, B_prev = p["ln_g"], p["ln_b"]
    for r in range(N_CORES):
        in_maps[r]["fin_g"] = _pack(G_prev)
        in_maps[r]["fin_b"] = _pack(B_prev)
        in_maps[r]["fin_grow"] = G_prev.astype(np.float32)[None, :]
    return in_maps


_NC_CACHE = None


def kernel(x, params):
    global _NC_CACHE
    if _NC_CACHE is None:
        _NC_CACHE = build_nc()
    in_maps = prep_in_maps(x, params)
    res = None
    last_err = None
    for _attempt in range(3):
        try:
            res = run_bass_kernel_spmd(_NC_CACHE, in_maps,
                                       core_ids=list(range(N_CORES)))
            break
        except Exception as e:  # transient device faults recover on retry
            last_err = e
    if res is None:
        raise last_err
    out_xT = res.results[0]["out_xT"]
    fin_b_host = np.asarray(params["local"]["ln_b"], np.float32)
    out = out_xT.T[None, :, :].astype(np.float32) + fin_b_host[None, None, :]
    return np.ascontiguousarray(out)


# revision 38
# speedup vs baseline: 1.1438x; 1.1438x over previous
"""Trainium2 Bass kernel for nn_AdvancedTransformer (axial + causal + local attention).

Strategy (8 NeuronCores, tensor-parallel over heads):
- Each core owns 2 of the 16 heads per attention layer.
- Data layout is "transposed": the carrier tensor xT is [D, S] so every
  projection matmul contracts D on the partition dim with no transposes.
- Flash-style softmax without max subtraction (scores are bounded ~|3.4|),
  row sums obtained for free via a ones-column appended to V in the P@V matmul.
- Causal/local masking via 0/1 mask multiplies after exp (host-precomputed),
  with band/triangle skipping of fully-masked work.
- Output projection produces per-core partial yT; an 8-core AllReduce (sum)
  combines them; residual + LayerNorm run replicated.
- LN gamma/beta are folded into the next layer's projection weights/biases on
  the host, so the on-device carrier is xhat (pre-affine LN output).
- Matmuls run in float32r (reduced-precision fp32, 4x faster; ~1e-4 rel err).
"""

import ml_dtypes
import numpy as np

import concourse.bass as bass
import concourse.mybir as mybir
import concourse.tile as tile
from concourse import bacc
from concourse.bass_utils import run_bass_kernel_spmd

F32 = mybir.dt.float32
F32R = mybir.dt.float32r
BF16 = mybir.dt.bfloat16
AF = mybir.ActivationFunctionType
ALU = mybir.AluOpType

B, S, D, H = 1, 2048, 1024, 16
HD = D // H          # 64
WIN = 256
N_CORES = 8
HPC = H // N_CORES   # heads per core = 2
P = 128
NKT = D // P         # 8 k-tiles over D
NST = S // P         # 16 s-tiles
NCH = S // 512       # 4 column chunks of 512
EPS = 1e-5
SCALE = float(1.0 / np.sqrt(HD))

LAYERS = [("ax", ["qr", "qc", "kr", "kc"]), ("ca", ["q", "k"]), ("lo", ["q", "k"])]


def _local_geom():
    """Per-stripe geometry for the local (banded) attention."""
    geo = []
    for i in range(NST):
        k0 = P * i
        qlo = min(max(k0 - 128, 0), S - 384)
        cmin, cmax = qlo // 512, (qlo + 383) // 512
        geo.append((qlo, cmin, cmax))
    contrib = {c: [i for i in range(NST) if geo[i][1] <= c <= geo[i][2]]
               for c in range(NCH)}
    return geo, contrib


def build_nc():
    nc = bacc.Bacc("TRN2", target_bir_lowering=False, debug=False,
                   num_devices=N_CORES)

    def din(name, shape, dt=F32R):
        return nc.dram_tensor(name, shape, dt, kind="ExternalInput").ap()

    # ---- inputs ----
    xT_d = din("xT", [D, S])
    ident_d = din("identity", [P, P], F32)
    tri_d = din("tri", [P, P], BF16)
    locmask_d = din("locmask", [NST, P, 384], BF16)
    ones_col_d = din("ones_col", [P, 1])
    ones128_d = din("ones128", [P, P])
    lay_in = {}
    for L, qks in LAYERS:
        for pn in qks + ["v"]:
            lay_in[f"{L}_W{pn}"] = din(f"{L}_W{pn}", [P, D])
        lay_in[f"{L}_pb"] = din(f"{L}_pb", [P, len(qks) + 1], F32)
        lay_in[f"{L}_Wo"] = din(f"{L}_Wo", [P, D])
        lay_in[f"{L}_bo"] = din(f"{L}_bo", [P, NKT], F32)
        lay_in[f"{L}_G"] = din(f"{L}_G", [P, NKT], F32)
        if L != "ax":
            lay_in[f"{L}_wsum"] = din(f"{L}_wsum", [1, (len(qks) + 1) * P])
            lay_in[f"{L}_negG"] = din(f"{L}_negG", [1, D])
    fin_g_d = din("fin_g", [P, NKT], F32)
    fin_b_d = din("fin_b", [P, NKT], F32)
    fin_grow_d = din("fin_grow", [1, D])

    out_d = nc.dram_tensor("out_xT", [D, S], F32, kind="ExternalOutput").ap()

    locgeo, loccontrib = _local_geom()

    with nc.allow_low_precision(reason="float32r matmul pipeline"), \
         tile.TileContext(nc) as tc:
        pX = tc.alloc_tile_pool(name="pX", bufs=NKT)
        pYF = tc.alloc_tile_pool(name="pYF", bufs=2)
        pQK = tc.alloc_tile_pool(name="pQK", bufs=4)
        pVT = tc.alloc_tile_pool(name="pVT", bufs=1)
        pPT = tc.alloc_tile_pool(name="pPT", bufs=2)
        pV = tc.alloc_tile_pool(name="pV", bufs=NST)
        pATT = tc.alloc_tile_pool(name="pATT", bufs=1)
        pW = tc.alloc_tile_pool(name="pW", bufs=3)
        pWO = tc.alloc_tile_pool(name="pWO", bufs=2)
        pMASK = tc.alloc_tile_pool(name="pMASK", bufs=3)
        pSQ = tc.alloc_tile_pool(name="pSQ", bufs=3)
        pYSB = tc.alloc_tile_pool(name="pYSB", bufs=3)
        pROW = tc.alloc_tile_pool(name="pROW", bufs=6)
        pZC = tc.alloc_tile_pool(name="pZC", bufs=10)
        pCONST = tc.alloc_tile_pool(name="pCONST", bufs=24)
        pPP = tc.alloc_tile_pool(name="pPP", bufs=4, space="PSUM")
        pOUT = tc.alloc_tile_pool(name="pOUT", bufs=1, space="PSUM")
        pDRAM = tc.alloc_tile_pool(name="pDRAM", bufs=2, space="DRAM")

        # ---- persistent loads ----
        X = []
        for i in range(NKT):
            xt = pX.tile([P, S], F32R, name=f"X{i}", tag="X")
            nc.sync.dma_start(out=xt[:, :], in_=xT_d[P * i:P * (i + 1), :])
            X.append(xt)
        ident = pCONST.tile([P, P], F32, name="ident")
        nc.sync.dma_start(out=ident[:, :], in_=ident_d[:, :])
        tri = pCONST.tile([P, P], BF16, name="tri")
        nc.sync.dma_start(out=tri[:, :], in_=tri_d[:, :])
        ones_col = pCONST.tile([P, 1], F32R, name="ones_col")
        nc.sync.dma_start(out=ones_col[:, :], in_=ones_col_d[:, :])
        ones128 = pCONST.tile([P, P], F32R, name="ones128")
        nc.sync.dma_start(out=ones128[:, :], in_=ones128_d[:, :])
        fin_g = pCONST.tile([P, NKT], F32, name="fin_g")
        nc.sync.dma_start(out=fin_g[:, :], in_=fin_g_d[:, :])
        fin_b = pCONST.tile([P, NKT], F32, name="fin_b")
        nc.sync.dma_start(out=fin_b[:, :], in_=fin_b_d[:, :])
        eps_t = pCONST.tile([1, 1], F32, name="eps_t")
        nc.vector.memset(eps_t[:, :], EPS)
        def proj_T(w_sb, bias_ap, out_sb, fold=None):
            """out_sb [128, S] = W^T @ carrier + b (transposed layout).
            With fold=(wsum_ap, mean_row, rstd_reps): carrier X holds z and
            the LN (z-mean)*rstd is folded in: psum = W^T z - wsum x mean,
            out = psum*rstd_rep + b."""
            for c in range(NCH):
                cs = slice(512 * c, 512 * (c + 1))
                ps = pPP.tile([P, 512], F32, tag="PP")
                last_kt = NKT - 1
                for kt in range(NKT):
                    nc.tensor.matmul(
                        ps[:, :], lhsT=w_sb[:, P * kt:P * (kt + 1)],
                        rhs=X[kt][:, cs],
                        start=(kt == 0),
                        stop=(fold is None and kt == last_kt))
                if fold is None:
                    nc.scalar.activation(out_sb[:, cs], ps[:, :],
                                         AF.Identity, bias=bias_ap, scale=1.0)
                else:
                    wsum_ap, mean_row, rstd_reps = fold
                    nc.tensor.matmul(ps[:, :], lhsT=wsum_ap,
                                     rhs=mean_row[0:1, cs],
                                     start=False, stop=True)
                    nc.vector.tensor_mul(out_sb[:, cs], ps[:, :],
                                         rstd_reps[c][:, :])
                    nc.vector.tensor_scalar_add(out_sb[:, cs], out_sb[:, cs],
                                                bias_ap)

        def row_attention(mode, qT, kT, v_tiles, h, attnT, layer_tag):
            """Standard attention for head h; writes normalized out into
            attnT[64h:64h+64, :]. mode in {"full", "causal", "local"}."""
            hp = slice(64 * h, 64 * h + 64)
            vcols = slice(65 * h, 65 * h + 65)  # [v_h | ones]
            outp = pOUT.tile([65, S], F32, tag="OUT",
                             name=f"{layer_tag}row{h}")
            for i in range(NST):
                k0 = P * i
                PT = pPT.tile([P, S], BF16, tag="PT")
                if mode == "full":
                    chunks = [(512 * c, 512 * (c + 1)) for c in range(NCH)]
                    pv_cs, first, last = list(range(NCH)), 0, NST - 1
                elif mode == "causal":
                    chunks = []
                    qs = k0
                    while qs < S:
                        qe = min((qs // 512 + 1) * 512, S)
                        chunks.append((qs, qe))
                        qs = qe
                    pv_cs = list(range(k0 // 512, NCH))
                else:  # local
                    qlo, cmin, cmax = locgeo[i]
                    chunks = [(qlo, qlo + 384)]
                    pv_cs = list(range(cmin, cmax + 1))
                for (qs, qe) in chunks:
                    sp = pPP.tile([P, qe - qs], F32, tag="PP")
                    nc.tensor.matmul(sp[:, :], lhsT=kT[hp, k0:k0 + P],
                                     rhs=qT[hp, qs:qe], start=True, stop=True)
                    nc.scalar.activation(PT[:, qs:qe], sp[:, :], AF.Exp,
                                         scale=SCALE)
                if mode == "causal":
                    nc.vector.tensor_mul(PT[:, k0:k0 + P], PT[:, k0:k0 + P],
                                         tri[:, :])
                    cb = 512 * (i // 4)
                    if k0 > cb:
                        nc.vector.memset(PT[:, cb:k0], 0.0)
                elif mode == "local":
                    qlo, cmin, cmax = locgeo[i]
                    nc.vector.tensor_mul(PT[:, qlo:qlo + 384],
                                         PT[:, qlo:qlo + 384],
                                         locmask_sb[:, 384 * i:384 * (i + 1)])
                    if qlo > 512 * cmin:
                        nc.vector.memset(PT[:, 512 * cmin:qlo], 0.0)
                    if qlo + 384 < 512 * (cmax + 1):
                        nc.vector.memset(PT[:, qlo + 384:512 * (cmax + 1)], 0.0)
                for c in pv_cs:
                    if mode == "full":
                        fi, la = 0, NST - 1
                    elif mode == "causal":
                        fi, la = 0, min(NST - 1, 4 * c + 3)
                    else:
                        fi, la = loccontrib[c][0], loccontrib[c][-1]
                    nc.tensor.matmul(outp[:, 512 * c:512 * (c + 1)],
                                     lhsT=v_tiles[i][:, vcols],
                                     rhs=PT[:, 512 * c:512 * (c + 1)],
                                     start=(i == fi), stop=(i == la))
            # normalize: out[hd, q] / Z[q]
            rz = pROW.tile([1, S], F32R, tag="ROW")
            nc.scalar.copy(rz[0:1, :], outp[64:65, :])
            nc.vector.reciprocal(rz[0:1, :], rz[0:1, :])
            for c in range(NCH):
                zrep = pPP.tile([64, 512], F32, tag="PP")
                nc.tensor.matmul(zrep[:, :], lhsT=ones128[0:1, 0:64],
                                 rhs=rz[0:1, 512 * c:512 * (c + 1)],
                                 start=True, stop=True)
                zrep_sb = pSQ.tile([64, 512], F32, tag="SQ")
                nc.scalar.copy(zrep_sb[:, :], zrep[:, :])
                nc.vector.tensor_mul(attnT[hp, 512 * c:512 * (c + 1)],
                                     outp[0:64, 512 * c:512 * (c + 1)],
                                     zrep_sb[:, :])

        def col_attention(qcT, kcT, v_tiles, h, attnT, layer_tag):
            """Axial column attention for head h; adds into attnT[64h:...]."""
            hp = slice(64 * h, 64 * h + 64)
            vdata = slice(65 * h, 65 * h + 64)  # v only (no ones col)
            colp = pOUT.tile([64, S], F32, tag="OUT", name=f"{layer_tag}col{h}")
            for r in range(NST):
                r0 = P * r
                PT = pPT.tile([P, S], BF16, tag="PT")
                zparts = []
                for c in range(NCH):
                    sp = pPP.tile([P, 512], F32, tag="PP")
                    nc.tensor.matmul(sp[:, :], lhsT=qcT[hp, r0:r0 + P],
                                     rhs=kcT[hp, 512 * c:512 * (c + 1)],
                                     start=True, stop=True)
                    zp = pZC.tile([P, 1], F32, tag="ZC")
                    nc.scalar.activation(PT[:, 512 * c:512 * (c + 1)], sp[:, :],
                                         AF.Exp, scale=SCALE, accum_out=zp[:, :])
                    zparts.append(zp)
                z01 = pZC.tile([P, 1], F32, tag="ZC")
                nc.vector.tensor_add(z01[:, :], zparts[0][:, :], zparts[1][:, :])
                z23 = pZC.tile([P, 1], F32, tag="ZC")
                nc.vector.tensor_add(z23[:, :], zparts[2][:, :], zparts[3][:, :])
                zs = pZC.tile([P, 1], F32, tag="ZC")
                nc.vector.tensor_add(zs[:, :], z01[:, :], z23[:, :])
                rzc = pZC.tile([P, 1], F32, tag="ZC")
                nc.vector.reciprocal(rzc[:, :], zs[:, :])
                vs = pZC.tile([P, 64], BF16, tag="VS", bufs=8)
                nc.vector.tensor_scalar_mul(vs[:, :], v_tiles[r][:, vdata],
                                            rzc[:, :])
                for c in range(NCH):
                    nc.tensor.matmul(colp[:, 512 * c:512 * (c + 1)],
                                     lhsT=vs[:, :],
                                     rhs=PT[:, 512 * c:512 * (c + 1)],
                                     start=(r == 0), stop=(r == NST - 1))
            for c in range(NCH):
                nc.vector.tensor_add(attnT[hp, 512 * c:512 * (c + 1)],
                                     attnT[hp, 512 * c:512 * (c + 1)],
                                     colp[:, 512 * c:512 * (c + 1)])

        # ================= layers =================
        # Carrier in X: raw x before layer 1, then z_n (pre-LN residual sum).
        # LN of z_{n-1} is folded into layer n's projections (mean via K=1
        # rank-1 matmul, rstd via epilogue scale); the residual x_{n-1} is
        # rebuilt during the X update: X = (G*X)*rstd_rep + yfull, where
        # yfull carries y + bo + B - G*row2 (constants injected on core 0).
        def emit_stats(L):
            mean_sb = pROW.tile([1, S], F32R, tag="ROW", name=f"{L}mean")
            msq_sb = pROW.tile([1, S], F32R, tag="ROW", name=f"{L}msq")
            wrow = pROW.tile([1, S], F32R, tag="ROW", name=f"{L}wrow")
            for c in range(NCH):
                cs = slice(512 * c, 512 * (c + 1))
                sps = pPP.tile([1, 512], F32, tag="PP", name=f"{L}sum{c}")
                for i in range(NKT):
                    nc.tensor.matmul(sps[:, :], lhsT=ones_col[:, :],
                                     rhs=X[i][:, cs], start=(i == 0),
                                     stop=(i == NKT - 1))
                nc.scalar.mul(mean_sb[0:1, cs], sps[:, :], 1.0 / D)
                sqs = pPP.tile([1, 512], F32, tag="PP", name=f"{L}sq{c}")
                for i in range(NKT):
                    sq = pSQ.tile([P, 512], F32R, tag="SQ")
                    eng = nc.gpsimd if i % 2 == 0 else nc.vector
                    eng.tensor_mul(sq[:, :], X[i][:, cs], X[i][:, cs])
                    nc.tensor.matmul(sqs[:, :], lhsT=ones_col[:, :],
                                     rhs=sq[:, :], start=(i == 0),
                                     stop=(i == NKT - 1))
                nc.scalar.mul(msq_sb[0:1, cs], sqs[:, :], 1.0 / D)
            nc.vector.tensor_mul(wrow[0:1, :], mean_sb[0:1, :], mean_sb[0:1, :])
            nc.vector.tensor_sub(msq_sb[0:1, :], msq_sb[0:1, :], wrow[0:1, :])
            nc.scalar.activation(wrow[0:1, :], msq_sb[0:1, :], AF.Sqrt,
                                 bias=eps_t[0:1, 0:1], scale=1.0)
            nc.vector.reciprocal(msq_sb[0:1, :], wrow[0:1, :])
            nc.vector.tensor_mul(wrow[0:1, :], mean_sb[0:1, :], msq_sb[0:1, :])
            rstd, row2 = msq_sb, wrow
            reps = []
            for c in range(NCH):
                cs = slice(512 * c, 512 * (c + 1))
                rp = pPP.tile([P, 512], F32, tag="PP", name=f"{L}rr{c}")
                nc.tensor.matmul(rp[:, :], lhsT=ones128[0:1, :],
                                 rhs=rstd[0:1, cs], start=True, stop=True)
                rs = pRREP.tile([P, 512], F32, tag="RREP", name=f"{L}rrs{c}")
                nc.scalar.copy(rs[:, :], rp[:, :])
                reps.append(rs)
            return mean_sb, reps, row2

        prev_stats = None
        for li, (L, qks) in enumerate(LAYERS):
            # -- load weights --
            wsb = {}
            for pn in qks + ["v"]:
                w = pW.tile([P, D], F32R, tag="W", name=f"{L}W{pn}")
                nc.sync.dma_start(out=w[:, :], in_=lay_in[f"{L}_W{pn}"][:, :])
                wsb[pn] = w
            wo = pWO.tile([P, D], F32R, tag="WO", name=f"{L}Wo")
            nc.sync.dma_start(out=wo[:, :], in_=lay_in[f"{L}_Wo"][:, :])
            pb = pCONST.tile([P, len(qks) + 1], F32, name=f"{L}pb")
            nc.sync.dma_start(out=pb[:, :], in_=lay_in[f"{L}_pb"][:, :])
            bo = pCONST.tile([P, NKT], F32, name=f"{L}bo")
            nc.sync.dma_start(out=bo[:, :], in_=lay_in[f"{L}_bo"][:, :])
            gg = pCONST.tile([P, NKT], F32, name=f"{L}G")
            nc.sync.dma_start(out=gg[:, :], in_=lay_in[f"{L}_G"][:, :])
            if prev_stats is not None:
                NP1 = len(qks) + 1
                wsum_sb = pCONST.tile([1, NP1 * P], F32R, name=f"{L}wsum")
                nc.sync.dma_start(out=wsum_sb[:, :],
                                  in_=lay_in[f"{L}_wsum"][:, :])
                negg_sb = pCONST.tile([1, D], F32R, name=f"{L}negG")
                nc.sync.dma_start(out=negg_sb[:, :],
                                  in_=lay_in[f"{L}_negG"][:, :])
                mean_sb, rstd_reps, row2 = prev_stats

            # -- projections (transposed, LN folded for layers 2+) --
            qkT = {}
            for j, pn in enumerate(qks):
                t = pQK.tile([P, S], BF16, tag="QK", name=f"{L}{pn}T")
                f = None if prev_stats is None else (
                    wsum_sb[0:1, P * j:P * (j + 1)], mean_sb, rstd_reps)
                proj_T(wsb[pn], pb[:, j:j + 1], t, fold=f)
                qkT[pn] = t
            # v: transposed projection then PE-transpose to natural
            jv = len(qks)
            vT = pVT.tile([P, S], F32, tag="VT", name=f"{L}vT")
            f = None if prev_stats is None else (
                wsum_sb[0:1, P * jv:P * (jv + 1)], mean_sb, rstd_reps)
            proj_T(wsb["v"], pb[:, jv:jv + 1], vT, fold=f)
            v_tiles = []
            for t in range(NST):
                pt = pPP.tile([P, P], F32, tag="PP", name=f"{L}vtr{t}")
                nc.tensor.transpose(pt[:, :], vT[:, P * t:P * (t + 1)],
                                    ident[:, :])
                vt = pV.tile([P, 65 * HPC], BF16, tag="V", name=f"{L}v{t}")
                for h in range(HPC):
                    nc.vector.tensor_copy(vt[:, 65 * h:65 * h + 64],
                                          pt[:, 64 * h:64 * h + 64])
                    nc.vector.memset(vt[:, 65 * h + 64:65 * h + 65], 1.0)
                v_tiles.append(vt)

            # -- attention --
            if L == "lo":
                locmask_sb = pMASK.tile([P, NST * 384], BF16, tag="MASK",
                                        name="locmask_sb")
                nc.sync.dma_start(
                    out=locmask_sb[:, :].rearrange("p (n q) -> p n q", n=NST),
                    in_=locmask_d[:, :, :].transpose([1, 0, 2]))
            attnT = pATT.tile([P, S], F32R, tag="ATT", name=f"{L}attnT")
            for h in range(HPC):
                if L == "ax":
                    row_attention("full", qkT["qr"], qkT["kr"], v_tiles, h,
                                  attnT, L)
                    col_attention(qkT["qc"], qkT["kc"], v_tiles, h, attnT, L)
                elif L == "ca":
                    row_attention("causal", qkT["q"], qkT["k"], v_tiles, h,
                                  attnT, L)
                else:
                    row_attention("local", qkT["q"], qkT["k"], v_tiles, h,
                                  attnT, L)

            # -- carrier transform (u-pass): X <- (G*X)*rstd_rep --
            # (turns z_{n-1} into x_{n-1} minus the constant terms, which
            # ride the collective below). Runs on DVE during attention/Wo.
            if prev_stats is not None:
                for i in range(NKT):
                    for c in range(NCH):
                        cs = slice(512 * c, 512 * (c + 1))
                        nc.vector.scalar_tensor_tensor(
                            out=X[i][:, cs], in0=X[i][:, cs],
                            scalar=gg[:, i:i + 1], in1=rstd_reps[c][:, :],
                            op0=ALU.mult, op1=ALU.mult)

            # -- output projection -> bf16 partial -> AllReduce --
            ybounce = pDRAM.tile([D, S], BF16, tag="YB", name=f"{L}yb")
            yfull = pDRAM.tile([D, S], BF16, tag="YFULL", name=f"{L}yf",
                               addr_space="Shared")
            for m in range(NKT):
                for c in range(NCH):
                    cs = slice(512 * c, 512 * (c + 1))
                    yp = pPP.tile([P, 512], F32, tag="PP")
                    nc.tensor.matmul(yp[:, :],
                                     lhsT=wo[:, P * m:P * (m + 1)],
                                     rhs=attnT[:, cs],
                                     start=True,
                                     stop=(prev_stats is None))
                    if prev_stats is not None:
                        # core-0 constant: -G (x) row2 (rank-1)
                        nc.tensor.matmul(yp[:, :],
                                         lhsT=negg_sb[0:1, P * m:P * (m + 1)],
                                         rhs=row2[0:1, cs],
                                         start=False, stop=True)
                    # PSUM->SBUF (bf16) + bias bo + B_prev (core 0)
                    y_sb = pYSB.tile([P, 512], BF16, tag="YSB")
                    if (m + c) % 2 == 0:
                        nc.vector.tensor_scalar_add(y_sb[:, :], yp[:, :],
                                                    bo[:, m:m + 1])
                    else:
                        nc.scalar.activation(y_sb[:, :], yp[:, :], AF.Identity,
                                             bias=bo[:, m:m + 1], scale=1.0)
                    nc.sync.dma_start(out=ybounce[P * m:P * (m + 1), cs],
                                      in_=y_sb[:, :])
            nc.gpsimd.collective_compute(
                "AllReduce", ALU.add,
                replica_groups=[list(range(N_CORES))],
                ins=[ybounce[:, :].opt()],
                outs=[yfull[:, :].opt()],
            )
            # -- X update: X <- X + yfull  (completes z_n) --
            for i in range(NKT):
                yf = pYF.tile([P, S], BF16, tag="YF")
                nc.sync.dma_start(out=yf[:, :],
                                  in_=yfull[P * i:P * (i + 1), :])
                nc.vector.tensor_add(X[i][:, :], X[i][:, :], yf[:, :])

            # -- stats of the new carrier z_n --
            prev_stats = emit_stats(L)

        # ================= final LN + affine =================
        # out = (z*rstd_rep)*g - g (x) row2  (B added on host), 2 DVE passes
        mean_sb, rstd_reps, row2 = prev_stats
        fin_grow = pW.tile([1, D], F32R, tag="W", name="fin_grow")
        nc.sync.dma_start(out=fin_grow[:, :], in_=fin_grow_d[:, :])
        for c in range(NCH):
            cs = slice(512 * c, 512 * (c + 1))
            for i in range(NKT):
                gr2 = pPP.tile([P, 512], F32, tag="PP", name=f"fg{i}_{c}")
                nc.tensor.matmul(gr2[:, :],
                                 lhsT=fin_grow[0:1, P * i:P * (i + 1)],
                                 rhs=row2[0:1, cs], start=True, stop=True)
                t1 = pSQ.tile([P, 512], F32, tag="SQ")
                nc.vector.tensor_mul(t1[:, :], X[i][:, cs],
                                     rstd_reps[c][:, :])
                nc.vector.scalar_tensor_tensor(
                    out=X[i][:, cs], in0=t1[:, :],
                    scalar=fin_g[:, i:i + 1], in1=gr2[:, :],
                    op0=ALU.mult, op1=ALU.subtract)
                nc.sync.dma_start(out=out_d[P * i:P * (i + 1), cs],
                                  in_=X[i][:, cs].bitcast(F32))

    nc.compile()
    return nc


def _pack(vec):
    """[1024] -> [128, 8] per-partition pack (col i = d-tile i)."""
    return np.ascontiguousarray(vec.reshape(NKT, P).T.astype(np.float32))


def _ktmajor(w):
    """[1024, 128] -> [128, 1024] with k-tile kt at cols [128kt:128kt+128)."""
    return np.ascontiguousarray(
        w.reshape(NKT, P, P).transpose(1, 0, 2).reshape(P, D).astype(np.float32))


def prep_in_maps(x, params):
    x = np.asarray(x, np.float32)
    pjax = {"ax": params["axial"], "ca": params["causal"], "lo": params["local"]}
    pp = {L: {k: np.asarray(v, np.float32) for k, v in pjax[L].items()}
          for L in pjax}

    idx = np.arange(S)
    common = {
        "xT": np.ascontiguousarray(x[0].T),
        "identity": np.eye(P, dtype=np.float32),
        "tri": (idx[:P, None] <= idx[None, :P]).astype(ml_dtypes.bfloat16),
        "ones_col": np.ones((P, 1), np.float32),
        "ones128": np.ones((P, P), np.float32),
    }
    locmask = np.zeros((NST, P, 384), np.float32)
    locgeo, _ = _local_geom()
    for i in range(NST):
        qlo, _, _ = locgeo[i]
        kk = P * i + idx[:P, None]
        qq = qlo + idx[None, :384]
        locmask[i] = (np.abs(kk - qq) <= WIN // 2).astype(np.float32)
    common["locmask"] = locmask.astype(ml_dtypes.bfloat16)

    in_maps = [dict(common) for _ in range(N_CORES)]
    G_prev = np.ones(D, np.float32)
    B_prev = np.zeros(D, np.float32)
    for L, qks in LAYERS:
        p = pp[L]
        names = {"ax": {"qr": "qr", "qc": "qc", "kr": "kr", "kc": "kc", "v": "v"},
                 "ca": {"q": "q", "k": "k", "v": "v"},
                 "lo": {"q": "q", "k": "k", "v": "v"}}[L]
        for r in range(N_CORES):
            hs = slice(r * P, (r + 1) * P)
            pbcols = []
            wsumcols = []
            for pn in qks + ["v"]:
                W = p["W" + names[pn]]
                bb = p["b" + names[pn]]
                Wf = W * G_prev[:, None]
                bf = bb + W.T @ B_prev
                in_maps[r][f"{L}_W{pn}"] = _ktmajor(Wf[:, hs])
                pbcols.append(bf[hs])
                wsumcols.append(-Wf[:, hs].sum(axis=0))
            in_maps[r][f"{L}_pb"] = np.ascontiguousarray(
                np.stack(pbcols, axis=1).astype(np.float32))
            in_maps[r][f"{L}_Wo"] = np.ascontiguousarray(p["Wo"][hs, :])
            bo_full = p["bo"] + B_prev
            in_maps[r][f"{L}_bo"] = (_pack(bo_full) if r == 0
                                     else np.zeros((P, NKT), np.float32))
            in_maps[r][f"{L}_G"] = _pack(G_prev)
            if L != "ax":
                in_maps[r][f"{L}_wsum"] = np.concatenate(
                    wsumcols).astype(np.float32)[None, :]
                in_maps[r][f"{L}_negG"] = ((-G_prev).astype(np.float32)[None, :]
                                           if r == 0
                                           else np.zeros((1, D), np.float32))
        G_prev, B_prev = p["ln_g"], p["ln_b"]
    for r in range(N_CORES):
        in_maps[r]["fin_g"] = _pack(G_prev)
        in_maps[r]["fin_b"] = _pack(B_prev)
        in_maps[r]["fin_grow"] = G_prev.astype(np.float32)[None, :]
    return in_maps


_NC_CACHE = None


def kernel(x, params):
    global _NC_CACHE
    if _NC_CACHE is None:
        _NC_CACHE = build_nc()
    in_maps = prep_in_maps(x, params)
    res = None
    last_err = None
    for _attempt in range(3):
        try:
            res = run_bass_kernel_spmd(_NC_CACHE, in_maps,
                                       core_ids=list(range(N_CORES)))
            break
        except Exception as e:  # transient device faults recover on retry
            last_err = e
    if res is None:
        raise last_err
    out_xT = res.results[0]["out_xT"]
    fin_b_host = np.asarray(params["local"]["ln_b"], np.float32)
    out = out_xT.T[None, :, :].astype(np.float32) + fin_b_host[None, None, :]
    return np.ascontiguousarray(out)
